# revision 38
# baseline (speedup 1.0000x reference)
"""Trainium2 Bass kernel for nn_IterativeFeatureExclusion.

Reference computation (B=4096, F=64, A=8, U=32, R=2):
    mask = 1 - eye(F)                       # [j, F]
    xm   = mask[:,None,:] * x[None,:,:]     # [j, B, F]
    z    = einsum('jbf,jafu->jabu', xm, K)  # [j, A, B, U]
    z    = softmax(z, axis=-1)
    w    = exp(K * 2)
    s    = einsum('jabu,jafu->jabf', z, w)  # [j, A, B, F]
    out  = softmax(s.mean(axis=0), axis=-1) # [A, B, F]

Key restructurings:
  * The input mask is folded into the weights: zeroing K[j,a,j,:] on the
    host makes the first einsum a plain matmul of x against a [F, A*F*U]
    matrix.
  * z is produced TRANSPOSED ([j/u on partitions, batch on free]) so both
    einsums contract on the partition axis; softmax-over-u group sums are
    computed on the TensorEngine with a block-diagonal ones matrix, which
    also broadcasts them back to all 32 u-lanes in the same matmul.
  * Softmax normalization (1/sum then multiply) runs as ONE custom DVE op
    per tile: exponent-flip seed + one Newton pass + multiply (registered
    at runtime; falls back to approx-reciprocal + tensor_mul).
  * The two K=64 z-matmuls of each j-tile pair run concurrently on
    disjoint PE row-groups; the scores matmuls of each (even, odd) "a"
    pair accumulate into disjoint col-groups of one PSUM tile and run
    concurrently as well.
  * The final softmax over f keeps f on partitions: ones-matrix matmul
    for the sum-broadcast + the same fused recip-multiply op. The output
    is written [A, F, B/8] per core and transposed on the host.
  * Both softmaxes skip max-subtraction (logits are O(+-8), exp is safe in
    fp32/bf16 range).
  * The 1/64 j-mean and the exp(2K) are folded into the host-prepped
    second-matmul weights; the j-sum rides the PSUM accumulator.

Sharding: data-parallel over batch, B/8 = 512 rows per NeuronCore; the
(~4 MB) kernel stack is replicated (sent as bf16, 2+2 MB).
"""

import numpy as np
import ml_dtypes
from contextlib import ExitStack

from concourse import bass, mybir
from concourse import tile
from concourse.bass_utils import run_bass_kernel_spmd
from concourse.dve_ops import RECIPROCAL_APPROX_FAST, RECIP_APPROX_FAST_CONSTS


def _ensure_fused_recip_mul():
    """Register a fused custom-DVE op: out = in1 * approx_recip(in0).

    Single streaming pass (6 ALU slices: exponent-flip seed, one
    Newton-Raphson refinement, multiply by in1) replacing the separate
    reciprocal + tensor_mul pair. ~0.17% max rel err on the reciprocal,
    inside this kernel's accuracy budget. Returns the DveOp, or None if
    registration fails (caller falls back to the two-op path).
    """
    import concourse.dve_ops as dve_ops

    name = "IFE_RECIP_MUL"
    try:
        for op in dve_ops.OPS:
            if op.name == name:
                return op
        import numpy as _np
        from concourse.dve_spec import Spec, Bin, AluOp, Src0, Src1, C0, C1, lower
        from concourse.dve_uop import DveOpSpec

        _not = Bin(AluOp.BITWISE_NOT, Src0, Src0)
        _y0 = _not * C0
        _y1 = _y0 * (C1 - Src0 * _y0)

        def _ref(in0, in1, s0, s1, imm2):
            not_x = (~in0.view(_np.int32)).view(_np.float32)
            y0 = not_x * s0
            y1 = y0 * (s1 - in0 * y0)
            return (y1 * in1).astype(_np.float32)

        spec = Spec(body=_y1 * Src1, reference=_ref)
        row = dve_ops._CUSTOM_DVE_ROW_BASE + len(dve_ops.OPS)
        if row >= 0x20:
            return None
        shas = {}
        for ver in ("v3", "v4"):
            uops = lower(spec, ver=ver)
            shas[ver] = DveOpSpec(
                name=name, opcode=row, uops=uops, rd1_en=True
            ).sha(ver)
        op = dve_ops.DveOp(name, spec, subdim=False, uops_sha=shas)
        dve_ops.OPS.append(op)
        dve_ops.CUSTOM_DVE_SPECS[name] = spec
        dve_ops._SUB_OPCODE_FOR_NAME[name] = row
        return op
    except Exception:
        return None

B, F, A, U = 4096, 64, 8, 32
RSCALE = 2.0
NCORES = 8
BC = B // NCORES          # 512 batch rows per core
JT, JG = 16, 4            # 16 j-tiles of 4 j's each (4*32u = 128 partitions)
NIDX = A * JT             # 128 (a, jt) tiles

F32 = mybir.dt.float32
BF16 = mybir.dt.bfloat16
EXP = mybir.ActivationFunctionType.Exp
DIV = mybir.AluOpType.divide
ADD = mybir.AluOpType.add

_CACHE: dict = {}


USE_FUSED_RECIP_MUL = True


def _kernel_body(tc, xt, k1, w2, bo, fon, out):
    nc = tc.nc
    fused = _ensure_fused_recip_mul() if USE_FUSED_RECIP_MUL else None
    with ExitStack() as ctx:
        singles = ctx.enter_context(tc.tile_pool(name="singles", bufs=1))
        # z and the group-sum pair share one pool: the sums reuse a z
        # buffer's banks once exp() has consumed it
        zp = ctx.enter_context(tc.tile_pool(name="zp", bufs=2, space="PSUM"))
        outp = ctx.enter_context(tc.tile_pool(name="outp", bufs=2, space="PSUM"))
        fsp = ctx.enter_context(tc.tile_pool(name="fsp", bufs=1, space="PSUM"))
        ep = ctx.enter_context(tc.tile_pool(name="ep", bufs=3))
        enp = ctx.enter_context(tc.tile_pool(name="enp", bufs=4))
        finp = ctx.enter_context(tc.tile_pool(name="finp", bufs=2))

        # DMA issue order matters (~600ns sequential issue per DMA on the
        # Sync queue): first-needed tensors go first, one DMA each
        xt_sb = singles.tile([128, BC], BF16)
        nc.sync.dma_start(xt_sb[:], xt[:])
        k1_t, w2_t = [], []
        for a in range(A):
            kt = singles.tile([128, 1024], BF16, tag=f"k1_{a}")
            k1_t.append(kt)
            wt = singles.tile([128, 1024], BF16, tag=f"w2_{a}")
            w2_t.append(wt)
        bo_sb = singles.tile([128, 128], BF16)
        fo_sb = singles.tile([128, 64], BF16)
        nc.sync.dma_start(k1_t[0][:], k1[:, 0:1024])
        nc.sync.dma_start(bo_sb[:], bo[:])
        nc.sync.dma_start(w2_t[0][:], w2[:, 0:1024])
        nc.sync.dma_start(fo_sb[:], fon[:])
        for a in range(1, A):
            sl = slice(a * 1024, (a + 1) * 1024)
            nc.sync.dma_start(k1_t[a][:], k1[:, sl])
            nc.sync.dma_start(w2_t[a][:], w2[:, sl])

        rc = RECIP_APPROX_FAST_CONSTS

        def u_softmax(a, jp):
            """z matmuls + exp + group-sum + normalize for one (a, jp).
            Returns the normalized-softmax tile en [128, 2*BC] (bf16)."""
            # z^T = K1^T @ x^T for two j-tiles (row-packed K=64);
            # tile_position puts them on disjoint row-groups so the PE
            # runs the pair concurrently.
            z = zp.tile([128, 2 * BC], F32, tag="zs")
            for h in range(2):
                col = jp * 128
                nc.tensor.matmul(
                    z[:, h * BC:(h + 1) * BC],
                    k1_t[a][h * 64:(h + 1) * 64, col:col + 128],
                    xt_sb[h * 64:(h + 1) * 64, :],
                    start=True, stop=True,
                    tile_position=(h * 64, 0),
                )
            # e = exp(z), one ACT pass over both tiles
            e = ep.tile([128, 2 * BC], BF16)
            nc.scalar.activation(e[:], z[:], EXP)
            en = enp.tile([128, 2 * BC], BF16)
            # group-sum + broadcast matmuls for both halves into one pair
            # tile, then a single FD=1024 fused normalize pass
            sb = zp.tile([128, 2 * BC], F32, tag="zs")
            for h in range(2):
                nc.tensor.matmul(
                    sb[:, h * BC:(h + 1) * BC], bo_sb[:],
                    e[:, h * BC:(h + 1) * BC],
                    start=True, stop=True,
                )
            with nc.allow_low_precision(reason="softmax recip; bf16 feeds bf16 matmul"):
                if fused is not None:
                    # en = e * approx_recip(sum) in ONE DVE pass
                    nc.vector._custom_dve(
                        fused, out=en[:], in0=sb[:], in1=e[:],
                        s0=rc["s0"], s1=rc["s1"],
                    )
                else:
                    rb = enp.tile([128, 2 * BC], BF16, tag="rb")
                    nc.vector._custom_dve(
                        RECIPROCAL_APPROX_FAST, out=rb[:], in0=sb[:],
                        s0=rc["s0"], s1=rc["s1"], imm2=rc["imm2"],
                    )
                    if (a * (JT // 2) + jp) % 4 == 0:
                        nc.vector.tensor_mul(en[:], e[:], rb[:])
                    else:
                        nc.gpsimd.tensor_mul(en[:], e[:], rb[:])
            return en

        for ap in range(A // 2):
            # a=2ap accumulates into partitions 0-63, a=2ap+1 into 64-127
            # (disjoint col-groups -> the PE runs each scores pair
            # concurrently); the two a's stay independent end-to-end
            out_ps = outp.tile([128, BC], F32)
            for jp in range(JT // 2):
                ens = [u_softmax(2 * ap + aoff, jp) for aoff in range(2)]
                for h in range(2):
                    jt = jp * 2 + h
                    for aoff in range(2):
                        nc.tensor.matmul(
                            out_ps[aoff * 64:(aoff + 1) * 64, :],
                            w2_t[2 * ap + aoff][:, jt * 64:(jt + 1) * 64],
                            ens[aoff][:, h * BC:(h + 1) * BC],
                            start=(jp == 0 and h == 0),
                            stop=(jp == JT // 2 - 1 and h == 1),
                            tile_position=(0, aoff * 64),
                            skip_group_check=True,
                        )
            # --- final softmax over f for both a's at once: f is on
            # partitions, so reuse the matmul-sum-broadcast + fused-recip-mul
            # pattern (output is written [f, b]; host transposes back) ---
            e2 = finp.tile([128, BC], BF16, tag="e2")
            nc.scalar.activation(e2[:], out_ps[:], EXP)
            fsum = fsp.tile([128, BC], F32)
            for aoff in range(2):
                sl = slice(aoff * 64, (aoff + 1) * 64)
                nc.tensor.matmul(
                    fsum[sl, :], fo_sb[sl, :], e2[sl, :],
                    start=True, stop=True,
                    tile_position=(aoff * 64, aoff * 64),
                    skip_group_check=True,
                )
            o2 = finp.tile([128, BC], F32, tag="o2")
            with nc.allow_low_precision(reason="final softmax via bf16 exp"):
                if fused is not None:
                    nc.vector._custom_dve(
                        fused, out=o2[:], in0=fsum[:], in1=e2[:],
                        s0=rc["s0"], s1=rc["s1"],
                    )
                else:
                    rb2 = finp.tile([128, BC], BF16, tag="rb2")
                    nc.vector._custom_dve(
                        RECIPROCAL_APPROX_FAST, out=rb2[:], in0=fsum[:],
                        s0=rc["s0"], s1=rc["s1"], imm2=rc["imm2"],
                    )
                    nc.vector.tensor_mul(o2[:], e2[:], rb2[:])
            for aoff in range(2):
                sl = slice(aoff * 64, (aoff + 1) * 64)
                nc.sync.dma_start(out[2 * ap + aoff], o2[sl, :])


def build_nc():
    from concourse.bacc import Bacc
    nc = Bacc()
    xt = nc.declare_dram_parameter("xt", [128, BC], BF16, isOutput=False)
    k1 = nc.declare_dram_parameter("k1", [128, 64 * 128], BF16, isOutput=False)
    w2 = nc.declare_dram_parameter("w2", [128, 128 * 64], BF16, isOutput=False)
    bo = nc.declare_dram_parameter("bones", [128, 128], BF16, isOutput=False)
    fon = nc.declare_dram_parameter("fones", [128, 64], BF16, isOutput=False)
    # output is [A, F, BC] on-device (contiguous [f, b] DMA per a);
    # the host transposes back to [A, BC, F]
    out = nc.declare_dram_parameter("out", [A, F, BC], F32, isOutput=True)
    with tile.TileContext(nc) as tc:
        _kernel_body(tc, xt, k1, w2, bo, fon, out)
    nc.compile()
    return nc


def prep_weights(kernels: np.ndarray):
    """Host-side packing of the (replicated) weight stack."""
    kf = kernels.astype(np.float32)
    km = kf.copy()
    km[np.arange(F), :, np.arange(F), :] = 0.0  # fold the exclusion mask

    # K1 blocks: [a, jt, f, (j_off, u)]
    t = km.transpose(1, 2, 0, 3).reshape(A, F, JT, JG, U)
    k1b = t.transpose(0, 2, 1, 3, 4).reshape(A, JT, F, JG * U)
    k1h = np.zeros((128, 64 * 128), dtype=np.float32)
    for idx in range(NIDX):
        a, jt = divmod(idx, JT)
        par = idx % 2
        col = (idx // 2) * 128
        k1h[par * 64:(par + 1) * 64, col:col + 128] = k1b[a, jt]

    # W2 blocks: [a, jt, (j_off, u), f], with exp(2K)/F folded in
    w = np.exp(RSCALE * kf) * (1.0 / F)
    w2b = w.transpose(1, 0, 3, 2).reshape(A, JT, JG, U, F).reshape(A, JT, JG * U, F)
    w2h = np.zeros((128, 128 * 64), dtype=np.float32)
    for idx in range(NIDX):
        a, jt = divmod(idx, JT)
        w2h[:, idx * 64:(idx + 1) * 64] = w2b[a, jt]

    # bones [128, 128]: block structure bones[k, m] = (k//32 == m//32)
    bones = np.kron(np.eye(JG, dtype=np.float32), np.ones((U, U), np.float32))
    # fones [128, 64]: all-ones for the final softmax's partition-sum matmul,
    # spanning both partition halves so base-64 slices match their operands
    fones = np.ones((128, 64), dtype=np.float32)

    bf = ml_dtypes.bfloat16
    return (k1h.astype(bf), w2h.astype(bf), bones.astype(bf), fones.astype(bf))


def prep_core_inputs(inputs: np.ndarray, kernels: np.ndarray):
    k1h, w2h, bones, fones = prep_weights(kernels)
    bf = ml_dtypes.bfloat16
    in_maps = []
    for c in range(NCORES):
        xs = inputs[c * BC:(c + 1) * BC, :].T.astype(np.float32)  # [64, BC]
        xth = np.concatenate([xs, xs], axis=0).astype(bf)         # [128, BC]
        in_maps.append({
            "xt": xth, "k1": k1h, "w2": w2h, "bones": bones, "fones": fones,
        })
    return in_maps


def gather_out(res) -> np.ndarray:
    """Gather per-core [A, F, BC] shards into the full [A, B, F] output."""
    shards = [np.asarray(res.results[c]["out"], dtype=np.float32)
              for c in range(NCORES)]
    full = np.concatenate(shards, axis=2)      # [A, F, B]
    return np.ascontiguousarray(full.transpose(0, 2, 1))


def _get_nc():
    if "nc" not in _CACHE:
        _CACHE["nc"] = build_nc()
    return _CACHE["nc"]


def kernel(inputs: np.ndarray, kernels: np.ndarray) -> np.ndarray:
    nc = _get_nc()
    in_maps = prep_core_inputs(np.asarray(inputs), np.asarray(kernels))
    res = run_bass_kernel_spmd(nc, in_maps, list(range(NCORES)))
    return gather_out(res)  # [A, B, F]



# revision 41
# speedup vs baseline: 1.7353x; 1.7353x over previous
"""Trainium2 Bass kernel for nn_IterativeFeatureExclusion.

Reference computation (B=4096, F=64, A=8, U=32, R=2):
    mask = 1 - eye(F)                       # [j, F]
    xm   = mask[:,None,:] * x[None,:,:]     # [j, B, F]
    z    = einsum('jbf,jafu->jabu', xm, K)  # [j, A, B, U]
    z    = softmax(z, axis=-1)
    w    = exp(K * 2)
    s    = einsum('jabu,jafu->jabf', z, w)  # [j, A, B, F]
    out  = softmax(s.mean(axis=0), axis=-1) # [A, B, F]

Key restructurings:
  * The input mask is folded into the weights: zeroing K[j,a,j,:] on the
    host makes the first einsum a plain matmul of x against a [F, A*F*U]
    matrix.
  * z is produced TRANSPOSED ([j/u on partitions, batch on free]) so both
    einsums contract on the partition axis; softmax-over-u group sums are
    computed on the TensorEngine with a block-diagonal ones matrix, which
    also broadcasts them back to all 32 u-lanes in the same matmul.
  * Softmax normalization (1/sum then multiply) runs as ONE custom DVE op
    per tile: exponent-flip seed + one Newton pass + multiply (registered
    at runtime; falls back to approx-reciprocal + tensor_mul).
  * The two K=64 z-matmuls of each j-tile pair run concurrently on
    disjoint PE row-groups; the scores matmuls of each (even, odd) "a"
    pair accumulate into disjoint col-groups of one PSUM tile and run
    concurrently as well.
  * The final softmax over f keeps f on partitions: ones-matrix matmul
    for the sum-broadcast + the same fused recip-multiply op. The output
    is written [A, F, B/8] per core and transposed on the host.
  * Both softmaxes skip max-subtraction (logits are O(+-8), exp is safe in
    fp32/bf16 range).
  * The 1/64 j-mean and the exp(2K) are folded into the host-prepped
    second-matmul weights; the j-sum rides the PSUM accumulator.

Sharding: data-parallel over batch, B/8 = 512 rows per NeuronCore; the
(~4 MB) kernel stack is replicated (sent as bf16, 2+2 MB).
"""

import numpy as np
import ml_dtypes
from contextlib import ExitStack

from concourse import bass, mybir
from concourse import tile
from concourse.bass_utils import run_bass_kernel_spmd
from concourse.dve_ops import RECIPROCAL_APPROX_FAST, RECIP_APPROX_FAST_CONSTS


def _ensure_fused_recip_mul():
    """Register a fused custom-DVE op: out = in1 * approx_recip(in0).

    Single streaming pass (6 ALU slices: exponent-flip seed, one
    Newton-Raphson refinement, multiply by in1) replacing the separate
    reciprocal + tensor_mul pair. ~0.17% max rel err on the reciprocal,
    inside this kernel's accuracy budget. Returns the DveOp, or None if
    registration fails (caller falls back to the two-op path).
    """
    import concourse.dve_ops as dve_ops

    name = "IFE_RECIP_MUL"
    try:
        for op in dve_ops.OPS:
            if op.name == name:
                return op
        import numpy as _np
        from concourse.dve_spec import Spec, Bin, AluOp, Src0, Src1, C0, C1, lower
        from concourse.dve_uop import DveOpSpec

        _not = Bin(AluOp.BITWISE_NOT, Src0, Src0)
        _y0 = _not * C0
        _y1 = _y0 * (C1 - Src0 * _y0)

        def _ref(in0, in1, s0, s1, imm2):
            not_x = (~in0.view(_np.int32)).view(_np.float32)
            y0 = not_x * s0
            y1 = y0 * (s1 - in0 * y0)
            return (y1 * in1).astype(_np.float32)

        spec = Spec(body=_y1 * Src1, reference=_ref)
        row = dve_ops._CUSTOM_DVE_ROW_BASE + len(dve_ops.OPS)
        if row >= 0x20:
            return None
        shas = {}
        for ver in ("v3", "v4"):
            uops = lower(spec, ver=ver)
            shas[ver] = DveOpSpec(
                name=name, opcode=row, uops=uops, rd1_en=True
            ).sha(ver)
        op = dve_ops.DveOp(name, spec, subdim=False, uops_sha=shas)
        dve_ops.OPS.append(op)
        dve_ops.CUSTOM_DVE_SPECS[name] = spec
        dve_ops._SUB_OPCODE_FOR_NAME[name] = row
        return op
    except Exception:
        return None

B, F, A, U = 4096, 64, 8, 32
RSCALE = 2.0
NCORES = 8
BC = B // NCORES          # 512 batch rows per core
JT, JG = 16, 4            # 16 j-tiles of 4 j's each (4*32u = 128 partitions)
NIDX = A * JT             # 128 (a, jt) tiles

F32 = mybir.dt.float32
BF16 = mybir.dt.bfloat16
EXP = mybir.ActivationFunctionType.Exp
DIV = mybir.AluOpType.divide
ADD = mybir.AluOpType.add

_CACHE: dict = {}


USE_FUSED_RECIP_MUL = True


def _kernel_body(tc, xt, k1, w2, bo, fon, out):
    nc = tc.nc
    fused = _ensure_fused_recip_mul() if USE_FUSED_RECIP_MUL else None
    with ExitStack() as ctx:
        singles = ctx.enter_context(tc.tile_pool(name="singles", bufs=1))
        zp = ctx.enter_context(tc.tile_pool(name="zp", bufs=2, space="PSUM"))
        sbp = ctx.enter_context(tc.tile_pool(name="sbp", bufs=2, space="PSUM"))
        outp = ctx.enter_context(tc.tile_pool(name="outp", bufs=1, space="PSUM"))
        fsp = ctx.enter_context(tc.tile_pool(name="fsp", bufs=1, space="PSUM"))
        ep = ctx.enter_context(tc.tile_pool(name="ep", bufs=3))
        enp = ctx.enter_context(tc.tile_pool(name="enp", bufs=4))
        finp = ctx.enter_context(tc.tile_pool(name="finp", bufs=2))

        # DMA issue order matters (~600ns sequential issue per DMA on the
        # Sync queue): first-needed tensors go first, one DMA each
        xt_sb = singles.tile([128, BC], BF16)
        nc.sync.dma_start(xt_sb[:], xt[:])
        k1_t, w2_t = [], []
        for a in range(A):
            kt = singles.tile([128, 1024], BF16, tag=f"k1_{a}")
            k1_t.append(kt)
            wt = singles.tile([128, 1024], BF16, tag=f"w2_{a}")
            w2_t.append(wt)
        bo_sb = singles.tile([128, 128], BF16)
        fo_sb = singles.tile([128, 64], BF16)
        nc.sync.dma_start(k1_t[0][:], k1[:, 0:1024])
        nc.sync.dma_start(bo_sb[:], bo[:])
        nc.sync.dma_start(w2_t[0][:], w2[:, 0:1024])
        nc.sync.dma_start(fo_sb[:], fon[:])
        for a in range(1, A):
            sl = slice(a * 1024, (a + 1) * 1024)
            nc.sync.dma_start(k1_t[a][:], k1[:, sl])
            nc.sync.dma_start(w2_t[a][:], w2[:, sl])

        rc = RECIP_APPROX_FAST_CONSTS

        def u_softmax(a, jp):
            """z matmuls + exp + group-sum + normalize for one (a, jp).
            Returns the normalized-softmax tile en [128, 2*BC] (bf16)."""
            # z^T = K1^T @ x^T for two j-tiles (row-packed K=64);
            # tile_position puts them on disjoint row-groups so the PE
            # runs the pair concurrently.
            z = zp.tile([128, 2 * BC], F32)
            for h in range(2):
                col = jp * 128
                nc.tensor.matmul(
                    z[:, h * BC:(h + 1) * BC],
                    k1_t[a][h * 64:(h + 1) * 64, col:col + 128],
                    xt_sb[h * 64:(h + 1) * 64, :],
                    start=True, stop=True,
                    tile_position=(h * 64, 0),
                )
            # e = exp(z), one ACT pass over both tiles
            e = ep.tile([128, 2 * BC], BF16)
            nc.scalar.activation(e[:], z[:], EXP)
            en = enp.tile([128, 2 * BC], BF16)
            rb = None
            if fused is None:
                rb = enp.tile([128, 2 * BC], BF16, tag="rb")
            for h in range(2):
                # group-sum + broadcast in one matmul
                sb = sbp.tile([128, BC], F32)
                nc.tensor.matmul(
                    sb[:], bo_sb[:], e[:, h * BC:(h + 1) * BC],
                    start=True, stop=True,
                )
                hs = slice(h * BC, (h + 1) * BC)
                with nc.allow_low_precision(reason="softmax recip; bf16 feeds bf16 matmul"):
                    if fused is not None:
                        # en = e * approx_recip(sum) in ONE DVE pass
                        nc.vector._custom_dve(
                            fused, out=en[:, hs], in0=sb[:], in1=e[:, hs],
                            s0=rc["s0"], s1=rc["s1"],
                        )
                    else:
                        nc.vector._custom_dve(
                            RECIPROCAL_APPROX_FAST, out=rb[:, hs], in0=sb[:],
                            s0=rc["s0"], s1=rc["s1"], imm2=rc["imm2"],
                        )
            if fused is None:
                if (a * (JT // 2) + jp) % 4 == 0:
                    nc.vector.tensor_mul(en[:], e[:], rb[:])
                else:
                    nc.gpsimd.tensor_mul(en[:], e[:], rb[:])
            return en

        for ap in range(A // 2):
            # a=2ap accumulates into partitions 0-63, a=2ap+1 into 64-127
            # (disjoint col-groups -> the PE runs each scores pair
            # concurrently); the two a's stay independent end-to-end
            out_ps = outp.tile([128, BC], F32)
            for jp in range(JT // 2):
                ens = [u_softmax(2 * ap + aoff, jp) for aoff in range(2)]
                for h in range(2):
                    jt = jp * 2 + h
                    for aoff in range(2):
                        nc.tensor.matmul(
                            out_ps[aoff * 64:(aoff + 1) * 64, :],
                            w2_t[2 * ap + aoff][:, jt * 64:(jt + 1) * 64],
                            ens[aoff][:, h * BC:(h + 1) * BC],
                            start=(jp == 0 and h == 0),
                            stop=(jp == JT // 2 - 1 and h == 1),
                            tile_position=(0, aoff * 64),
                            skip_group_check=True,
                        )
            # --- final softmax over f for both a's at once: f is on
            # partitions, so reuse the matmul-sum-broadcast + fused-recip-mul
            # pattern (output is written [f, b]; host transposes back) ---
            e2 = finp.tile([128, BC], BF16, tag="e2")
            nc.scalar.activation(e2[:], out_ps[:], EXP)
            fsum = fsp.tile([128, BC], F32)
            for aoff in range(2):
                sl = slice(aoff * 64, (aoff + 1) * 64)
                nc.tensor.matmul(
                    fsum[sl, :], fo_sb[sl, :], e2[sl, :],
                    start=True, stop=True,
                    tile_position=(aoff * 64, aoff * 64),
                    skip_group_check=True,
                )
            o2 = finp.tile([128, BC], F32, tag="o2")
            with nc.allow_low_precision(reason="final softmax via bf16 exp"):
                if fused is not None:
                    nc.vector._custom_dve(
                        fused, out=o2[:], in0=fsum[:], in1=e2[:],
                        s0=rc["s0"], s1=rc["s1"],
                    )
                else:
                    rb2 = finp.tile([128, BC], BF16, tag="rb2")
                    nc.vector._custom_dve(
                        RECIPROCAL_APPROX_FAST, out=rb2[:], in0=fsum[:],
                        s0=rc["s0"], s1=rc["s1"], imm2=rc["imm2"],
                    )
                    nc.vector.tensor_mul(o2[:], e2[:], rb2[:])
            for aoff in range(2):
                sl = slice(aoff * 64, (aoff + 1) * 64)
                nc.sync.dma_start(out[2 * ap + aoff], o2[sl, :])


def build_nc():
    from concourse.bacc import Bacc
    nc = Bacc()
    xt = nc.declare_dram_parameter("xt", [128, BC], BF16, isOutput=False)
    k1 = nc.declare_dram_parameter("k1", [128, 64 * 128], BF16, isOutput=False)
    w2 = nc.declare_dram_parameter("w2", [128, 128 * 64], BF16, isOutput=False)
    bo = nc.declare_dram_parameter("bones", [128, 128], BF16, isOutput=False)
    fon = nc.declare_dram_parameter("fones", [128, 64], BF16, isOutput=False)
    # output is [A, F, BC] on-device (contiguous [f, b] DMA per a);
    # the host transposes back to [A, BC, F]
    out = nc.declare_dram_parameter("out", [A, F, BC], F32, isOutput=True)
    with tile.TileContext(nc) as tc:
        _kernel_body(tc, xt, k1, w2, bo, fon, out)
    nc.compile()
    return nc


def prep_weights(kernels: np.ndarray):
    """Host-side packing of the (replicated) weight stack."""
    kf = kernels.astype(np.float32)
    km = kf.copy()
    km[np.arange(F), :, np.arange(F), :] = 0.0  # fold the exclusion mask

    # K1 blocks: [a, jt, f, (j_off, u)]
    t = km.transpose(1, 2, 0, 3).reshape(A, F, JT, JG, U)
    k1b = t.transpose(0, 2, 1, 3, 4).reshape(A, JT, F, JG * U)
    k1h = np.zeros((128, 64 * 128), dtype=np.float32)
    for idx in range(NIDX):
        a, jt = divmod(idx, JT)
        par = idx % 2
        col = (idx // 2) * 128
        k1h[par * 64:(par + 1) * 64, col:col + 128] = k1b[a, jt]

    # W2 blocks: [a, jt, (j_off, u), f], with exp(2K)/F folded in
    w = np.exp(RSCALE * kf) * (1.0 / F)
    w2b = w.transpose(1, 0, 3, 2).reshape(A, JT, JG, U, F).reshape(A, JT, JG * U, F)
    w2h = np.zeros((128, 128 * 64), dtype=np.float32)
    for idx in range(NIDX):
        a, jt = divmod(idx, JT)
        w2h[:, idx * 64:(idx + 1) * 64] = w2b[a, jt]

    # bones [128, 128]: block structure bones[k, m] = (k//32 == m//32)
    bones = np.kron(np.eye(JG, dtype=np.float32), np.ones((U, U), np.float32))
    # fones [128, 64]: all-ones for the final softmax's partition-sum matmul,
    # spanning both partition halves so base-64 slices match their operands
    fones = np.ones((128, 64), dtype=np.float32)

    bf = ml_dtypes.bfloat16
    return (k1h.astype(bf), w2h.astype(bf), bones.astype(bf), fones.astype(bf))


def prep_core_inputs(inputs: np.ndarray, kernels: np.ndarray):
    k1h, w2h, bones, fones = prep_weights(kernels)
    bf = ml_dtypes.bfloat16
    in_maps = []
    for c in range(NCORES):
        xs = inputs[c * BC:(c + 1) * BC, :].T.astype(np.float32)  # [64, BC]
        xth = np.concatenate([xs, xs], axis=0).astype(bf)         # [128, BC]
        in_maps.append({
            "xt": xth, "k1": k1h, "w2": w2h, "bones": bones, "fones": fones,
        })
    return in_maps


def gather_out(res) -> np.ndarray:
    """Gather per-core [A, F, BC] shards into the full [A, B, F] output."""
    shards = [np.asarray(res.results[c]["out"], dtype=np.float32)
              for c in range(NCORES)]
    full = np.concatenate(shards, axis=2)      # [A, F, B]
    return np.ascontiguousarray(full.transpose(0, 2, 1))


def _get_nc():
    if "nc" not in _CACHE:
        _CACHE["nc"] = build_nc()
    return _CACHE["nc"]


def kernel(inputs: np.ndarray, kernels: np.ndarray) -> np.ndarray:
    nc = _get_nc()
    in_maps = prep_core_inputs(np.asarray(inputs), np.asarray(kernels))
    res = run_bass_kernel_spmd(nc, in_maps, list(range(NCORES)))
    return gather_out(res)  # [A, B, F]

